# revision 16
# baseline (speedup 1.0000x reference)
"""AgentCrossAttention Trainium2 kernel.

Math (per frame (b,t), 512 frames total):
    q = rmsnorm(agent @ Wq) * q_gamma            (16 heads x 64)
    k = rmsnorm(z @ Wk) * k_gamma                (256 tokens x 4 kv-heads x 64)
    v = z @ Wv
    scores = (q . k) * hd^-0.5 ; softcapped tanh ; softmax over s
    out = attn @ v ; y = out @ Wo

Sharding: data-parallel over the 512 (b,t) frames -> 64 frames per core.

Device-side design notes:
  - Everything bf16 into the PE array, fp32 PSUM accumulation.
  - z is cast to bf16 AND transposed on the host (input marshalling), so the
    contraction dim D sits on SBUF partitions straight off a plain DMA.
  - k is produced kv-major (k^T) with Wk stationary; v is produced
    token-major with z^T stationary (needed as attn@v rhs).
  - rmsnorm over the head dim (which sits on partitions for k^T/q^T) is done
    with a block-diag(64x64 ones) matmul that leaves the per-head sum of
    squares replicated across each 64-partition block.  The k-side block
    matrix carries 1/(q_gamma*k_gamma)^2 so both gammas ride through the
    rsqrt for free; the attention scale 1/sqrt(64) is folded into the tanh
    softcap scale.
  - Wq columns / Wo rows are host-permuted so that the per-kv-head q slices,
    the attention output, and the final projection all line up on partitions
    with zero on-device shuffles.  head h <-> (half=(h//4)%2, m=(h//8)*4+h%4),
    new index = m*128 + half*64 + d.
  - softmax runs on full [128,256] tiles (only rows 32j+g are real; garbage
    rows are never read downstream).
  - attn^T (needed for attn@v lhsT) is two 128x128 PE transposes.
"""

import numpy as np
import ml_dtypes

import concourse.bass as bass
import concourse.bacc as bacc
import concourse.mybir as mybir
import concourse.tile as tile
from concourse.bass_utils import run_bass_kernel_spmd

F32 = mybir.dt.float32
BF16 = mybir.dt.bfloat16
AF = mybir.ActivationFunctionType
AX = mybir.AxisListType

DIM = 1024
H = 16
HKV = 4
HD = 64
G = 4
B, T, S = 4, 128, 256
NCORES = 8
FPC = (B * T) // NCORES          # frames per core = 64
KT = DIM // 128                  # 8 contraction tiles
SOFT_CAP = 50.0
SCALE = HD ** -0.5


def _head_colmap():
    """new index m*128 + half*64 + d  <-  old index h*64 + d."""
    cm = np.empty(DIM, dtype=np.int64)
    for m in range(8):
        for half in range(2):
            j = (m // 4) * 2 + half
            g = m % 4
            h = 4 * j + g
            for d in range(HD):
                cm[m * 128 + half * 64 + d] = h * 64 + d
    return cm


def _build_bass():
    nc = bacc.Bacc("TRN2", target_bir_lowering=False, debug=False)
    z = nc.dram_tensor("z", [FPC, DIM, S], BF16, kind="ExternalInput")
    agT = nc.dram_tensor("agT", [128, KT, FPC], BF16, kind="ExternalInput")
    wq = nc.dram_tensor("wq", [128, KT, 8, 128], BF16, kind="ExternalInput")
    wk = nc.dram_tensor("wk", [128, KT, 256], BF16, kind="ExternalInput")
    wv = nc.dram_tensor("wv", [128, KT, 256], BF16, kind="ExternalInput")
    wo = nc.dram_tensor("wo", [128, KT, DIM], BF16, kind="ExternalInput")
    ones = nc.dram_tensor("ones", [128, 128], BF16, kind="ExternalInput")
    onesg = nc.dram_tensor("onesg", [128, 128], BF16, kind="ExternalInput")
    ident = nc.dram_tensor("ident", [128, 128], BF16, kind="ExternalInput")
    yT = nc.dram_tensor("yT", [128, KT, FPC], F32, kind="ExternalOutput")

    with tile.TileContext(nc) as tc:
        with (
            tc.tile_pool(name="const", bufs=1) as cpool,
            tc.tile_pool(name="persist", bufs=1) as ppool,
            tc.tile_pool(name="qsb", bufs=1) as qsb,
            tc.tile_pool(name="zt", bufs=8) as zpool,
            tc.tile_pool(name="fsb", bufs=3) as fsb,
            tc.tile_pool(name="atp", bufs=3) as atp,
            tc.tile_pool(name="psk", bufs=2, space="PSUM") as psk,
            tc.tile_pool(name="psv", bufs=2, space="PSUM") as psv,
            tc.tile_pool(name="pss", bufs=1, space="PSUM") as pss,
            tc.tile_pool(name="psc", bufs=1, space="PSUM") as psc,
            tc.tile_pool(name="pso", bufs=1, space="PSUM") as pso,
            tc.tile_pool(name="psa", bufs=1, space="PSUM") as psa,
        ):
            wq_sb = cpool.tile([128, KT, 8, 128], BF16)
            nc.sync.dma_start(wq_sb[:], wq[:])
            wk_sb = cpool.tile([128, KT, 256], BF16)
            nc.sync.dma_start(wk_sb[:], wk[:])
            wv_sb = cpool.tile([128, KT, 256], BF16)
            nc.sync.dma_start(wv_sb[:], wv[:])
            wo_sb = cpool.tile([128, KT, DIM], BF16)
            nc.sync.dma_start(wo_sb[:], wo[:])
            ones_sb = cpool.tile([128, 128], BF16)
            nc.sync.dma_start(ones_sb[:], ones[:])
            onesg_sb = cpool.tile([128, 128], BF16)
            nc.sync.dma_start(onesg_sb[:], onesg[:])
            ident_sb = cpool.tile([128, 128], BF16)
            nc.sync.dma_start(ident_sb[:], ident[:])
            agT_sb = cpool.tile([128, KT, FPC], BF16)
            nc.sync.dma_start(agT_sb[:], agT[:])

            qgT_sb = ppool.tile([128, 8, FPC], BF16)   # normalized q^T (permuted)
            oT_sb = ppool.tile([128, KT, FPC], BF16)   # attention out^T (permuted)

            # ---------------- q projection + rmsnorm (once) ----------------
            psum_q = psk.tile([128, 8, FPC], F32, tag="k")
            for m in range(8):
                for kt in range(KT):
                    nc.tensor.matmul(
                        psum_q[:, m, :],
                        wq_sb[:, kt, m, :],
                        agT_sb[:, kt, :],
                        start=(kt == 0),
                        stop=(kt == KT - 1),
                    )
            qsq = qsb.tile([128, 8, FPC], BF16)
            nc.scalar.square(qsq[:], psum_q[:])
            psum_qs = psv.tile([128, 8, FPC], F32, tag="v")
            for m in range(8):
                nc.tensor.matmul(
                    psum_qs[:, m, :], ones_sb[:], qsq[:, m, :],
                    start=True, stop=True,
                )
            qinv = qsb.tile([128, 8, FPC], F32)
            nc.vector.reciprocal(qinv[:], psum_qs[:])
            qrs = qsb.tile([128, 8, FPC], F32)
            nc.scalar.activation(qrs[:], qinv[:], AF.Sqrt, scale=float(HD))
            nc.vector.tensor_mul(qgT_sb[:], psum_q[:], qrs[:])

            # ---------------- per-frame attention ----------------
            for f in range(FPC):
                zT = zpool.tile([128, KT, S], BF16, tag="zT")
                nc.sync.dma_start(
                    zT[:], z[f].rearrange("(kt p) s -> p kt s", p=128))

                # k^T (kv-major), Wk stationary
                psum_k = psk.tile([128, 2, S], F32, tag="k")
                for kvt in range(2):
                    for kt in range(KT):
                        nc.tensor.matmul(
                            psum_k[:, kvt, :],
                            wk_sb[:, kt, kvt * 128:(kvt + 1) * 128],
                            zT[:, kt, :],
                            start=(kt == 0),
                            stop=(kt == KT - 1),
                        )
                # v (token-major), z^T stationary
                psum_v = psv.tile([128, 2, 256], F32, tag="v")
                for st in range(2):
                    for kt in range(KT):
                        nc.tensor.matmul(
                            psum_v[:, st, :],
                            zT[:, kt, st * 128:(st + 1) * 128],
                            wv_sb[:, kt, :],
                            start=(kt == 0),
                            stop=(kt == KT - 1),
                        )
                v_sb = fsb.tile([128, 2, 256], BF16, tag="v_sb")
                nc.scalar.copy(v_sb[:], psum_v[:])

                # rmsnorm(k): sumsq replicated over each 64-part block;
                # onesg carries 1/(q_gamma*k_gamma)^2 so krs = g2*rsqrt(mean)
                ksq = fsb.tile([128, 2, S], BF16, tag="ksq")
                nc.scalar.square(ksq[:], psum_k[:])
                psum_ss = pss.tile([128, 2, S], F32, tag="ss")
                for kvt in range(2):
                    nc.tensor.matmul(
                        psum_ss[:, kvt, :], onesg_sb[:], ksq[:, kvt, :],
                        start=True, stop=True,
                    )
                kinv = fsb.tile([128, 2, S], F32, tag="kinv")
                nc.vector.reciprocal(kinv[:], psum_ss[:])
                krs = fsb.tile([128, 2, S], F32, tag="krs")
                nc.scalar.activation(krs[:], kinv[:], AF.Sqrt, scale=float(HD))
                knT = fsb.tile([128, 2, S], BF16, tag="knT")
                nc.vector.tensor_mul(knT[:], psum_k[:], krs[:])

                # scores: per kv-head j, [4,256] at psum partitions 32j
                comb = psc.tile([128, 264], F32, tag="sc")  # scores + outT
                for j in range(HKV):
                    h2 = 64 * (j % 2)
                    mc = (j // 2) * 4
                    nc.tensor.matmul(
                        comb[32 * j:32 * j + 4, 0:256],
                        qgT_sb[h2:h2 + 64, mc:mc + 4, f],
                        knT[h2:h2 + 64, j // 2, :],
                        start=True, stop=True,
                        tile_position=(h2, 32 * j),
                    )
                # softcap tanh + softmax along s (free dim)
                tcap = fsb.tile([128, S], F32, tag="tcap")
                nc.scalar.activation(
                    tcap[:], comb[:, 0:256], AF.Tanh, scale=SCALE / SOFT_CAP
                )
                mx = fsb.tile([128, 1], F32, tag="mx")
                nc.vector.reduce_max(mx[:], tcap[:], axis=AX.X)
                b50 = fsb.tile([128, 1], F32, tag="b50")
                nc.vector.tensor_scalar_mul(b50[:], mx[:], -SOFT_CAP)
                esb = fsb.tile([128, S], F32, tag="esb")
                ssum = fsb.tile([128, 1], F32, tag="ssum")
                nc.scalar.activation(
                    esb[:], tcap[:], AF.Exp,
                    bias=b50[:], scale=SOFT_CAP, accum_out=ssum[:],
                )
                rinv = fsb.tile([128, 1], F32, tag="rinv")
                nc.vector.reciprocal(rinv[:], ssum[:])
                attn_sb = fsb.tile([128, S], BF16, tag="attn")
                nc.vector.tensor_mul(attn_sb[:], esb[:],
                                     rinv[:].broadcast_to((128, S)))

                # attn^T via PE transpose (2 x 128x128)
                psum_at = psa.tile([128, 2, 128], BF16, tag="at")
                for st in range(2):
                    nc.tensor.transpose(
                        psum_at[:, st, :],
                        attn_sb[:, st * 128:(st + 1) * 128],
                        ident_sb[:],
                    )
                attnT = atp.tile([128, 2, 128], BF16, tag="attnT")
                nc.vector.tensor_copy(attnT[:], psum_at[:])

                # attn @ v  ->  out^T [64 hd, 4 g] per j (permuted layout)
                for j in range(HKV):
                    pb = 64 * (j % 2)
                    mc = (j // 2) * 4
                    for st in range(2):
                        nc.tensor.matmul(
                            comb[pb:pb + 64, 256 + mc:256 + mc + 4],
                            v_sb[:, st, j * 64:(j + 1) * 64],
                            attnT[:, st, 32 * j:32 * j + 4],
                            start=(st == 0),
                            stop=(st == 1),
                        )
                nc.vector.tensor_copy(oT_sb[:, :, f], comb[:, 256:264])

            # ---------------- output projection (once) ----------------
            psum_y = pso.tile([128, KT, FPC], F32, tag="y")
            for dt in range(KT):
                for kt in range(KT):
                    nc.tensor.matmul(
                        psum_y[:, dt, :],
                        wo_sb[:, kt, dt * 128:(dt + 1) * 128],
                        oT_sb[:, kt, :],
                        start=(kt == 0),
                        stop=(kt == KT - 1),
                    )
            y_sb = qsb.tile([128, KT, FPC], F32, tag="ysb")
            nc.vector.tensor_copy(y_sb[:], psum_y[:])
            nc.gpsimd.dma_start(yT[:], y_sb[:])

    nc.compile()
    return nc


_NC_CACHE = {}
last_results = None


def kernel(agent_tokens, z_tokens, Wq, Wk, Wv, Wo, q_gamma, k_gamma):
    global last_results
    bf = ml_dtypes.bfloat16

    agent = np.ascontiguousarray(np.asarray(agent_tokens, np.float32)).reshape(
        B * T, DIM)
    zfull = np.asarray(z_tokens, np.float32).reshape(B * T, S, DIM)
    Wq = np.asarray(Wq, np.float32)
    Wk = np.asarray(Wk, np.float32)
    Wv = np.asarray(Wv, np.float32)
    Wo = np.asarray(Wo, np.float32)
    q_gamma = np.asarray(q_gamma, np.float32)
    k_gamma = np.asarray(k_gamma, np.float32)

    cm = _head_colmap()
    wq_host = np.ascontiguousarray(
        Wq[:, cm].reshape(KT, 128, 8, 128).transpose(1, 0, 2, 3)).astype(bf)
    wk_host = np.ascontiguousarray(
        Wk.reshape(KT, 128, 256).transpose(1, 0, 2)).astype(bf)
    wv_host = np.ascontiguousarray(
        Wv.reshape(KT, 128, 256).transpose(1, 0, 2)).astype(bf)
    wo_host = np.ascontiguousarray(
        Wo[cm, :].reshape(KT, 128, DIM).transpose(1, 0, 2)).astype(bf)
    blk = np.zeros((128, 128), np.float32)
    blk[:64, :64] = 1.0
    blk[64:, 64:] = 1.0
    ones_host = blk.astype(bf)
    g2v = np.tile((q_gamma * k_gamma).astype(np.float32), 2)      # (128,)
    onesg_host = (blk / (g2v ** 2)[None, :]).astype(bf)
    ident_host = np.eye(128, dtype=np.float32).astype(bf)

    if "nc" not in _NC_CACHE:
        _NC_CACHE["nc"] = _build_bass()
    nc = _NC_CACHE["nc"]

    in_maps = []
    for c in range(NCORES):
        fr = slice(c * FPC, (c + 1) * FPC)
        agT_host = np.ascontiguousarray(
            agent[fr].T.reshape(KT, 128, FPC).transpose(1, 0, 2)).astype(bf)
        # z host-transposed: [f, D, S] so D lands on SBUF partitions
        z_host = np.ascontiguousarray(
            zfull[fr].astype(bf).transpose(0, 2, 1))
        in_maps.append({
            "z": z_host, "agT": agT_host, "wq": wq_host, "wk": wk_host,
            "wv": wv_host, "wo": wo_host, "ones": ones_host,
            "onesg": onesg_host, "ident": ident_host,
        })

    res = run_bass_kernel_spmd(nc, in_maps, core_ids=list(range(NCORES)))
    last_results = res

    outs = []
    for c in range(NCORES):
        yT = np.asarray(res.results[c]["yT"], np.float32)   # [128, KT, FPC]
        outs.append(yT.transpose(2, 1, 0).reshape(FPC, DIM))
    return np.concatenate(outs, axis=0).reshape(B, T, DIM).astype(np.float32)
